# revision 48
# baseline (speedup 1.0000x reference)
"""Longformer attention (B=1, S=4096, D=512, H=8, HD=64, window=512, nglobal=64)
on 8 Trainium2 NeuronCores, head-parallel (core c computes head c).

v6 layout strategy (per core):
  - All matmul operands bf16 (psum accumulation fp32). Host pre-transposes
    inputs to xT [512, 4096] bf16 and packs wq|wk|wv, bq|bk|bv, mask tables.
  - DMA issue parallelism: sync issues xq chunk 0 then the projection
    weights then the remaining xq chunks; scalar issues the xkv chunks;
    gpsimd issues the small late-use constants.
  - Projections transposed: qT/kT [128(d_sw|d_g), 4096]; v transposed to
    natural layout vcomb [s%128, kt, {sw,g}, d|1] via PE transposes one
    chunk behind. kTz/qTgz are zero-padded copies so every attention matmul
    is nominally 128-contract (keeps the PE HAM clock gate warm; padding
    rows multiply against zeros).
  - Global-query/global-key exp work (Eg per pair, B logit groups) runs
    during phase A: pair T's Eg needs only q-chunk T, B group g needs only
    k-chunks <= 2g+1. This leaves phase C's ACT with sliding-window exps
    only. B's AV accumulation runs at the end of phase A.
  - Sliding-window attention over 512-query supertile PAIRS (8 pairs),
    two-stage pipeline logits(T) | AV(T-1). Slots are packed into 2-bank
    psum groups with 512-col bank-aligned subslots (a matmul output must
    not cross a psum bank). Triangle masks are 128x128 table multiplies on
    gpsimd. AV starts with the full-width slot 4 so start=True covers the
    whole psum tile.
  - No on-device out-projection or normalization: the device exports
    xTall [65, 4096] (unnormalized AV outputs + ones-row denominators) and
    the host computes sum_h (x_h/den_h) @ wo_h + b_out.
"""
import os
import sys
import functools

for _p in ("/opt/trn_rl_repo",):
    if os.path.isdir(_p) and _p not in sys.path:
        sys.path.insert(0, _p)

import numpy as np
from ml_dtypes import bfloat16

import concourse.bass as bass
import concourse.tile as tile
from concourse import bacc, mybir
from concourse.bass_utils import run_bass_kernel_spmd

S = 4096
F = 512          # d_model
HD = 64          # head dim
H = 8
WIN = 512        # sliding window (left 256, right 256)
PT = 512         # query supertile pair
NPT = S // PT    # 8
KT = 128         # key tile
NKT = S // KT    # 32
N_CORES = 8
F32 = mybir.dt.float32
BF16 = mybir.dt.bfloat16

# per-slot stored query ranges (slot s covers ktile 4T-2+s, k_rel=128(s-2)+kk)
_SLOT_W = {0: 128, 1: 256, 2: 384, 3: 512, 4: 512, 5: 384, 6: 256, 7: 128}
_SLOT_QOFF = {0: 0, 1: 0, 2: 0, 3: 0, 4: 0, 5: 128, 6: 256, 7: 384}
# mask table column offsets
_TBL_A = 0      # qq' <= kk-1   (upper band edge)
_TBL_B = 128    # qq' >= kk     (lower band edge)
_TBL_G2 = 256   # A | (kk < ng)
_TBL_GO = 384   # kk < ng only


def _build_tbl(ng: int):
    kk = np.arange(KT)[:, None]
    qq = np.arange(KT)[None, :]
    A = (qq <= kk - 1).astype(np.float32)
    B = (qq >= kk).astype(np.float32)
    G2 = np.maximum(A, (kk < ng).astype(np.float32) * np.ones_like(A))
    GO = ((kk < ng).astype(np.float32) * np.ones_like(A))
    return np.concatenate([A, B, G2, GO], axis=1).astype(bfloat16)  # [128, 512]


def _pair_slots(T: int, ng: int):
    """[(s, ktile, width, qoff)] for pair T."""
    s_lo = 2 if T == 0 else 0
    s_hi = 6 if T == NPT - 1 else 8
    out = []
    for s in range(s_lo, s_hi):
        j = 4 * T - 2 + s
        w = _SLOT_W[s]
        if T == 0 and s == 2 and ng > 0:
            w = 512          # extend ktile-0 range so all queries see global keys
        out.append((s, j, w, _SLOT_QOFF[s]))
    return out


def _pack_pair(T: int, ng: int):
    """Pack slots into 2-bank psum groups with 512-col bank-aligned subslots
    (a matmul output may not cross a psum bank boundary).

    Returns (groups, emap): groups = [(placements, width, ebase)] with
    placements = [(slot, pos)]; emap[s] = column of slot s in the E tile.
    """
    slots = _pair_slots(T, ng)
    rem = sorted(slots, key=lambda x: -x[2])
    subslots = []
    while rem:
        big = rem.pop(0)
        sub, fill = [(big, 0)], big[2]
        while fill < 512:
            nxt = next((x for x in rem if x[2] <= 512 - fill), None)
            if nxt is None:
                break
            rem.remove(nxt)
            sub.append((nxt, fill))
            fill += nxt[2]
        subslots.append((sub, fill))
    groups, emap, ebase = [], {}, 0
    for i in range(0, len(subslots), 2):
        chunk = subslots[i:i + 2]
        placements, width = [], 0
        for gi, (sub, fill) in enumerate(chunk):
            base = gi * 512
            for (sl, off) in sub:
                assert off + sl[2] <= 512
                placements.append((sl, base + off))
                emap[sl[0]] = ebase + base + off
            width = base + fill
        groups.append((placements, width, ebase))
        ebase += width
    return groups, emap


def _mask_ops(T: int, slots, emap, ng: int):
    """[(ecol, width, tbl_off)] mask multiplies into the E tile."""
    ops = []
    for (s, j, w, qoff) in slots:
        e = emap[s]
        if s == 0:
            ops.append((e + 0, 128, _TBL_A))
        elif s == 1:
            ops.append((e + 128, 128, _TBL_A))
        elif s == 2:
            if T == 0 and ng > 0:
                ops.append((e + 256, 128, _TBL_G2))
                ops.append((e + 384, 128, _TBL_GO))
            else:
                ops.append((e + 256, 128, _TBL_A))
        elif s == 3:
            ops.append((e + 384, 128, _TBL_A))
        else:  # 4, 5, 6, 7
            ops.append((e + 0, 128, _TBL_B))
    return ops


def _build_program(ng: int):
    nc = bacc.Bacc("TRN2", target_bir_lowering=False, debug=False,
                   num_devices=N_CORES)

    d = {}
    d["xqT"] = nc.dram_tensor("xqT", [F, S], BF16, kind="ExternalInput").ap()
    d["xkvT"] = nc.dram_tensor("xkvT", [F, S], BF16, kind="ExternalInput").ap()
    d["wqkv"] = nc.dram_tensor("wqkv", [F, 3 * 128], BF16, kind="ExternalInput").ap()
    d["b3"] = nc.dram_tensor("b3", [128, 3], F32, kind="ExternalInput").ap()
    d["tbl"] = nc.dram_tensor("tbl", [128, 512], BF16, kind="ExternalInput").ap()
    d["ident"] = nc.dram_tensor("ident", [128, 128], BF16, kind="ExternalInput").ap()
    xall_ap = nc.dram_tensor("xall", [HD + 1, S], BF16, kind="ExternalOutput").ap()
    xg_ap = (nc.dram_tensor("xg", [HD + 1, ng], BF16, kind="ExternalOutput").ap()
             if ng > 0 else None)

    SC = 512            # projection s-chunk (== PT)
    NSC = S // SC       # 8
    FT = F // 128       # 4 f-chunks
    kpg = max(1, 512 // max(ng, 1))          # B-logit ktiles per psum group
    nbg = (NKT + kpg - 1) // kpg if ng else 0

    Exp = mybir.ActivationFunctionType.Exp

    with tile.TileContext(nc) as tc:
        with (
            tc.tile_pool(name="const", bufs=1) as constp,
            tc.tile_pool(name="big", bufs=1) as bigp,
        ):
            # ---- constants: critical ones early on sync, the rest on gpsimd ----
            wqkv_sb = constp.tile([128, FT, 3 * 128], BF16, tag="wqkv")
            b3_sb = constp.tile([128, 3], F32, tag="b3")
            tbl_sb = constp.tile([128, 512], BF16, tag="tbl")
            nc.gpsimd.dma_start(tbl_sb[:], d["tbl"][:])
            id_sb = constp.tile([128, 128], BF16, tag="id")
            nc.gpsimd.dma_start(id_sb[:], d["ident"][:])

            # ---- full-input prefetch: sync->xq chunks, scalar->xkv chunks ----
            xq_sb = bigp.tile([128, FT, S], BF16, tag="xq")
            xkv_sb = bigp.tile([128, FT, S], BF16, tag="xkv")
            xqr = d["xqT"].rearrange("(c p) s -> p c s", p=128)
            xkvr = d["xkvT"].rearrange("(c p) s -> p c s", p=128)
            # first chunk small for a fast start, then 1024-col chunks
            # (2KB per-partition-row descriptors for full DMA efficiency)
            xchunks = [(0, 512), (512, 1024), (1536, 1024), (2560, 1024),
                       (3584, 512)]
            nc.sync.dma_start(xq_sb[:, :, 0:SC], xqr[:, :, 0:SC])
            nc.sync.dma_start(wqkv_sb[:],
                              d["wqkv"].rearrange("(c p) e -> p c e", p=128))
            nc.sync.dma_start(b3_sb[:], d["b3"][:])
            for cs, cw in xchunks[1:]:
                nc.sync.dma_start(xq_sb[:, :, cs:cs + cw], xqr[:, :, cs:cs + cw])
            for cs, cw in xchunks:
                nc.scalar.dma_start(xkv_sb[:, :, cs:cs + cw],
                                    xkvr[:, :, cs:cs + cw])

            qT = bigp.tile([128, S], BF16, tag="qT")     # rows 0:64 sw, 64:128 g
            kT = bigp.tile([128, S], BF16, tag="kT")
            # zero-padded variants for nominally-128-contract attention matmuls
            kTz = bigp.tile([128, S], BF16, tag="kTz")   # rows 0:64 k_sw, 64:128 zero
            nc.vector.memset(kTz[64:128, :], 0.0)
            if ng > 0:
                qTgz = bigp.tile([128, 128], BF16, tag="qTgz")  # rows 64:128 q_g
                nc.gpsimd.memset(qTgz[0:64, :], 0.0)
            # v natural: [s%128, ktile, {sw,g}, d|ones]
            vcomb = bigp.tile([128, NKT, 2, HD + 1], BF16, tag="vcomb")
            nc.vector.memset(vcomb[:, :, :, HD], 1.0)
            # unnormalized attention outputs (transposed) + denominators row 64
            xTall = bigp.tile([HD + 1, NPT, PT], BF16, tag="xTall")
            # global-key exp weights per pair (1..7), rows ng:128 kept zero
            if ng > 0:
                Egbuf = bigp.tile([128, NPT - 1, PT], BF16, tag="Egbuf")
                nc.gpsimd.memset(Egbuf[ng:128, :, :], 0.0)
                egB = bigp.tile([128, NKT, ng], BF16, tag="egB")
                xgB = bigp.tile([HD + 1, ng], BF16, tag="xgB")

            # ============ Phase A: projections + global-attention prep ==========
            with (
                tc.tile_pool(name="vtmp", bufs=2) as vtmpp,
                tc.tile_pool(name="pa", bufs=4, space="PSUM") as pap,
                tc.tile_pool(name="ptr", bufs=3, space="PSUM") as ptrp,
                tc.tile_pool(name="pbx", bufs=1, space="PSUM") as pbxp,
            ):
                def a_proj(sc):
                    ss = sc * SC
                    pq = pap.tile([128, SC], F32, tag="pa")
                    for ft in range(FT):
                        nc.tensor.matmul(pq[:], wqkv_sb[:, ft, 0:128],
                                         xq_sb[:, ft, ss:ss + SC],
                                         start=(ft == 0), stop=(ft == FT - 1))
                    nc.vector.tensor_scalar_add(qT[:, ss:ss + SC], pq[:],
                                                b3_sb[:, 0:1])
                    pk = pap.tile([128, SC], F32, tag="pa")
                    for ft in range(FT):
                        nc.tensor.matmul(pk[:], wqkv_sb[:, ft, 128:256],
                                         xkv_sb[:, ft, ss:ss + SC],
                                         start=(ft == 0), stop=(ft == FT - 1))
                    nc.vector.tensor_scalar_add(kT[:, ss:ss + SC], pk[:],
                                                b3_sb[:, 1:2])
                    nc.vector.tensor_scalar_add(kTz[0:64, ss:ss + SC],
                                                pk[0:64, :], b3_sb[0:64, 1:2])
                    if ng > 0 and sc == 0:
                        nc.vector.tensor_scalar_add(qTgz[64:128, 0:ng],
                                                    pq[64:128, 0:ng],
                                                    b3_sb[64:128, 0:1])
                    pv = pap.tile([128, SC], F32, tag="pa")
                    for ft in range(FT):
                        nc.tensor.matmul(pv[:], wqkv_sb[:, ft, 256:384],
                                         xkv_sb[:, ft, ss:ss + SC],
                                         start=(ft == 0), stop=(ft == FT - 1))
                    vt = vtmpp.tile([128, SC], BF16, tag="vt")
                    nc.vector.tensor_scalar_add(vt[:], pv[:], b3_sb[:, 2:3])
                    return vt

                def a_transpose(sc, vt):
                    for sb in range(SC // 128):
                        kt_idx = sc * (SC // 128) + sb
                        ptr = ptrp.tile([128, 128], BF16, tag="tr")
                        nc.tensor.transpose(ptr[:], vt[:, sb * 128:(sb + 1) * 128],
                                            id_sb[:])
                        src = ptr[:].rearrange("p (b x) -> p b x", b=2)
                        dst = vcomb[:, kt_idx, :, 0:HD]
                        if kt_idx % 2 == 0:
                            nc.vector.tensor_copy(dst, src)
                        else:
                            nc.scalar.copy(dst, src)

                def a_eg(T):
                    # global-key logits+exp for pair T (needs q chunk T only)
                    qs = T * PT
                    plg = pap.tile([ng, PT], F32, tag="pa")
                    nc.tensor.matmul(plg[:], kTz[:, 0:ng], qT[:, qs:qs + PT],
                                     start=True, stop=True)
                    nc.scalar.activation(Egbuf[0:ng, T - 1, :], plg[:],
                                         Exp, scale=0.125)

                def a_bgroup(g):
                    # global-query logits+exp for ktiles [g*kpg, (g+1)*kpg)
                    nkt_g = min(kpg, NKT - g * kpg)
                    plB = pap.tile([128, nkt_g * ng], F32, tag="pa")
                    for i in range(nkt_g):
                        kt = g * kpg + i
                        nc.tensor.matmul(plB[:, i * ng:(i + 1) * ng],
                                         kT[:, kt * KT:(kt + 1) * KT],
                                         qTgz[:, 0:ng],
                                         start=True, stop=True)
                    nc.scalar.activation(
                        egB[:, g * kpg:g * kpg + nkt_g, :],
                        plB[:].rearrange("p (a b) -> p a b", a=nkt_g),
                        Exp, scale=0.125)

                bstate = {}

                def b_av_chunk(c):
                    # AV over ktiles [8c, 8c+8): needs egB group c (after
                    # sc=2c+1) and vcomb ktiles (transposed after sc=2c+1)
                    if c == 0:
                        pxg = pbxp.tile([HD + 1, ng], F32, tag="BX")
                        bstate["pxg"] = pxg
                    pxg = bstate["pxg"]
                    for kt in range(8 * c, 8 * c + 8):
                        nc.tensor.matmul(pxg[:], vcomb[:, kt, 1, :],
                                         egB[:, kt, :],
                                         start=(kt == 0), stop=(kt == NKT - 1),
                                         skip_group_check=True)

                prev = None
                for sc in range(NSC):
                    vt = a_proj(sc)
                    if prev is not None:
                        a_transpose(sc - 1, prev)
                    prev = vt
                    if ng:
                        if sc >= 1:
                            a_eg(sc)
                        if sc % 2 == 1 and (sc - 1) // 2 < nbg:
                            a_bgroup((sc - 1) // 2)
                        if sc in (5, 7):
                            b_av_chunk(sc - 5 if sc == 5 else 1)
                a_transpose(NSC - 1, prev)
                if ng:
                    for g in range(4, nbg):
                        a_bgroup(g)
                    b_av_chunk(2)
                    b_av_chunk(3)
                    nc.vector.tensor_copy(xgB[:], bstate["pxg"][:])
                    nc.sync.dma_start(xg_ap[:], xgB[:])

            # ============ Phase C: paired sliding-window attention ==============
            with (
                tc.tile_pool(name="E", bufs=2) as ep,
                tc.tile_pool(name="pL", bufs=3, space="PSUM") as pLp,
                tc.tile_pool(name="pX", bufs=2, space="PSUM") as pXp,
            ):
                stash = {}

                def stage_L(T):
                    qs = T * PT
                    slots = _pair_slots(T, ng)
                    groups, emap = _pack_pair(T, ng)
                    E = ep.tile([128, 2560], BF16, tag="E")
                    for (placements, gw, ebase) in groups:
                        pl = pLp.tile([128, gw], F32, tag="L")
                        for ((s, j, w, qoff), pos) in placements:
                            nc.tensor.matmul(pl[:, pos:pos + w],
                                             kTz[:, j * KT:(j + 1) * KT],
                                             qT[:, qs + qoff:qs + qoff + w],
                                             start=True, stop=True)
                        nc.scalar.activation(E[:, ebase:ebase + gw], pl[:],
                                             Exp, scale=0.125)
                    # triangle masks (split between gpsimd and DVE)
                    for mi, (ecol, w, toff) in enumerate(_mask_ops(T, slots,
                                                                  emap, ng)):
                        eng = nc.gpsimd if mi % 2 == 0 else nc.vector
                        eng.tensor_mul(E[:, ecol:ecol + w],
                                       E[:, ecol:ecol + w],
                                       tbl_sb[:, toff:toff + w])
                    stash[T] = (E, emap)

                def stage_AV(T):
                    E, emap = stash.pop(T)
                    slots = _pair_slots(T, ng)
                    # s4 goes first: full [0,512) range (its start=True covers
                    # the whole psum tile) and its mask is ready earliest
                    ordered = ([sl for sl in slots if sl[0] == 4] +
                               [sl for sl in slots if sl[0] != 4])
                    has_g = ng > 0 and T >= 1
                    px2 = pXp.tile([HD + 1, PT], F32, tag="X")
                    n = len(ordered)
                    for idx, (s, j, w, qoff) in enumerate(ordered):
                        e = emap[s]
                        nc.tensor.matmul(px2[:, qoff:qoff + w],
                                         vcomb[:, j, 0, :], E[:, e:e + w],
                                         start=(idx == 0),
                                         stop=(idx == n - 1 and not has_g),
                                         skip_group_check=True)
                    if has_g:
                        nc.tensor.matmul(px2[:], vcomb[:, 0, 0, :],
                                         Egbuf[:, T - 1, :],
                                         start=False, stop=True,
                                         skip_group_check=True)
                    nc.vector.tensor_copy(xTall[:, T, :], px2[:])
                    nc.sync.dma_start(xall_ap[:, T * PT:(T + 1) * PT],
                                      xTall[:, T, :])

                for step in range(NPT + 1):
                    if step < NPT:
                        stage_L(step)
                    if step >= 1:
                        stage_AV(step - 1)

    nc.compile()
    return nc


@functools.lru_cache(maxsize=4)
def _get_program(ng: int):
    return _build_program(ng)


def kernel(inputs_q, inputs_kv, global_mask,
           w_q_sw, b_q_sw, w_k_sw, b_k_sw, w_v_sw, b_v_sw,
           w_q_g, b_q_g, w_k_g, b_k_g, w_v_g, b_v_g,
           w_out, b_out,
           _trace=False, _tmpdir=None):
    gm = np.asarray(global_mask[0]).astype(bool)
    ng = int(gm.sum())
    assert gm[:ng].all() and not gm[ng:].any(), "global_mask must be a prefix mask"
    assert ng <= 128, "kernel specialized for ng <= 128"

    xqT = np.ascontiguousarray(np.asarray(inputs_q[0], np.float32).T).astype(bfloat16)
    xkvT = np.ascontiguousarray(np.asarray(inputs_kv[0], np.float32).T).astype(bfloat16)
    tbl = _build_tbl(ng)
    ident = np.eye(128, dtype=bfloat16)

    nc = _get_program(ng)

    in_maps = []
    for h in range(N_CORES):
        wq = np.concatenate([w_q_sw[:, h, :], w_q_g[:, h, :]], axis=1)
        wk = np.concatenate([w_k_sw[:, h, :], w_k_g[:, h, :]], axis=1)
        wv = np.concatenate([w_v_sw[:, h, :], w_v_g[:, h, :]], axis=1)
        wqkv = np.concatenate([wq, wk, wv], axis=1).astype(bfloat16)
        b3 = np.stack([np.concatenate([b_q_sw[h], b_q_g[h]]).reshape(-1),
                       np.concatenate([b_k_sw[h], b_k_g[h]]).reshape(-1),
                       np.concatenate([b_v_sw[h], b_v_g[h]]).reshape(-1)],
                      axis=1).astype(np.float32)
        in_maps.append({
            "xqT": xqT, "xkvT": xkvT,
            "wqkv": wqkv, "b3": b3,
            "tbl": tbl, "ident": ident,
        })

    res = run_bass_kernel_spmd(nc, in_maps, list(range(N_CORES)),
                               trace=_trace, tmpdir=_tmpdir)
    out = np.zeros((S, F), np.float32)
    for h in range(N_CORES):
        xall = np.asarray(res.results[h]["xall"], dtype=np.float32)  # [65, S]
        xh = (xall[:HD] / xall[HD]).T                                # [S, 64]
        if ng > 0:
            xg = np.asarray(res.results[h]["xg"], dtype=np.float32)  # [65, ng]
            xh[:ng] = (xg[:HD] / xg[HD]).T
        out += xh @ np.asarray(w_out[h], np.float32)
    out += np.asarray(b_out, np.float32)
    if _trace:
        kernel._last_results = res
    return out[None].astype(np.float32)


# revision 51
# speedup vs baseline: 1.0070x; 1.0070x over previous
"""Longformer attention (B=1, S=4096, D=512, H=8, HD=64, window=512, nglobal=64)
on 8 Trainium2 NeuronCores, head-parallel (core c computes head c).

v6 layout strategy (per core):
  - All matmul operands bf16 (psum accumulation fp32). Host pre-transposes
    inputs to xT [512, 4096] bf16 and packs wq|wk|wv, bq|bk|bv, mask tables.
  - DMA issue parallelism: sync issues xq chunk 0 then the projection
    weights then the remaining xq chunks; scalar issues the xkv chunks;
    gpsimd issues the small late-use constants.
  - Projections transposed: qT/kT [128(d_sw|d_g), 4096]; v transposed to
    natural layout vcomb [s%128, kt, {sw,g}, d|1] via PE transposes one
    chunk behind. kTz/qTgz are zero-padded copies so every attention matmul
    is nominally 128-contract (keeps the PE HAM clock gate warm; padding
    rows multiply against zeros).
  - Global-query/global-key exp work (Eg per pair, B logit groups) runs
    during phase A: pair T's Eg needs only q-chunk T, B group g needs only
    k-chunks <= 2g+1. This leaves phase C's ACT with sliding-window exps
    only. B's AV accumulation runs at the end of phase A.
  - Sliding-window attention over 512-query supertile PAIRS (8 pairs),
    two-stage pipeline logits(T) | AV(T-1). Slots are packed into 2-bank
    psum groups with 512-col bank-aligned subslots (a matmul output must
    not cross a psum bank). Triangle masks are 128x128 table multiplies on
    gpsimd. AV starts with the full-width slot 4 so start=True covers the
    whole psum tile.
  - No on-device out-projection or normalization: the device exports
    xTall [65, 4096] (unnormalized AV outputs + ones-row denominators) and
    the host computes sum_h (x_h/den_h) @ wo_h + b_out.
"""
import os
import sys
import functools

for _p in ("/opt/trn_rl_repo",):
    if os.path.isdir(_p) and _p not in sys.path:
        sys.path.insert(0, _p)

import numpy as np
from ml_dtypes import bfloat16

import concourse.bass as bass
import concourse.tile as tile
from concourse import bacc, mybir
from concourse.bass_utils import run_bass_kernel_spmd

S = 4096
F = 512          # d_model
HD = 64          # head dim
H = 8
WIN = 512        # sliding window (left 256, right 256)
PT = 512         # query supertile pair
NPT = S // PT    # 8
KT = 128         # key tile
NKT = S // KT    # 32
N_CORES = 8
F32 = mybir.dt.float32
BF16 = mybir.dt.bfloat16

# per-slot stored query ranges (slot s covers ktile 4T-2+s, k_rel=128(s-2)+kk)
_SLOT_W = {0: 128, 1: 256, 2: 384, 3: 512, 4: 512, 5: 384, 6: 256, 7: 128}
_SLOT_QOFF = {0: 0, 1: 0, 2: 0, 3: 0, 4: 0, 5: 128, 6: 256, 7: 384}
# mask table column offsets
_TBL_A = 0      # qq' <= kk-1   (upper band edge)
_TBL_B = 128    # qq' >= kk     (lower band edge)
_TBL_G2 = 256   # A | (kk < ng)
_TBL_GO = 384   # kk < ng only


def _build_tbl(ng: int):
    kk = np.arange(KT)[:, None]
    qq = np.arange(KT)[None, :]
    A = (qq <= kk - 1).astype(np.float32)
    B = (qq >= kk).astype(np.float32)
    G2 = np.maximum(A, (kk < ng).astype(np.float32) * np.ones_like(A))
    GO = ((kk < ng).astype(np.float32) * np.ones_like(A))
    return np.concatenate([A, B, G2, GO], axis=1).astype(bfloat16)  # [128, 512]


def _pair_slots(T: int, ng: int):
    """[(s, ktile, width, qoff)] for pair T."""
    s_lo = 2 if T == 0 else 0
    s_hi = 6 if T == NPT - 1 else 8
    out = []
    for s in range(s_lo, s_hi):
        j = 4 * T - 2 + s
        w = _SLOT_W[s]
        if T == 0 and s == 2 and ng > 0:
            w = 512          # extend ktile-0 range so all queries see global keys
        out.append((s, j, w, _SLOT_QOFF[s]))
    return out


def _pack_pair(T: int, ng: int):
    """Pack slots into 2-bank psum groups with 512-col bank-aligned subslots
    (a matmul output may not cross a psum bank boundary).

    Returns (groups, emap): groups = [(placements, width, ebase)] with
    placements = [(slot, pos)]; emap[s] = column of slot s in the E tile.
    """
    slots = _pair_slots(T, ng)
    rem = sorted(slots, key=lambda x: -x[2])
    subslots = []
    while rem:
        big = rem.pop(0)
        sub, fill = [(big, 0)], big[2]
        while fill < 512:
            nxt = next((x for x in rem if x[2] <= 512 - fill), None)
            if nxt is None:
                break
            rem.remove(nxt)
            sub.append((nxt, fill))
            fill += nxt[2]
        subslots.append((sub, fill))
    groups, emap, ebase = [], {}, 0
    for i in range(0, len(subslots), 3):
        chunk = subslots[i:i + 3]
        placements, width = [], 0
        for gi, (sub, fill) in enumerate(chunk):
            base = gi * 512
            for (sl, off) in sub:
                assert off + sl[2] <= 512
                placements.append((sl, base + off))
                emap[sl[0]] = ebase + base + off
            width = base + fill
        groups.append((placements, width, ebase))
        ebase += width
    return groups, emap


def _mask_ops(T: int, slots, emap, ng: int):
    """[(ecol, width, tbl_off)] mask multiplies into the E tile."""
    ops = []
    for (s, j, w, qoff) in slots:
        e = emap[s]
        if s == 0:
            ops.append((e + 0, 128, _TBL_A))
        elif s == 1:
            ops.append((e + 128, 128, _TBL_A))
        elif s == 2:
            if T == 0 and ng > 0:
                ops.append((e + 256, 128, _TBL_G2))
                ops.append((e + 384, 128, _TBL_GO))
            else:
                ops.append((e + 256, 128, _TBL_A))
        elif s == 3:
            ops.append((e + 384, 128, _TBL_A))
        else:  # 4, 5, 6, 7
            ops.append((e + 0, 128, _TBL_B))
    return ops


def _build_program(ng: int):
    nc = bacc.Bacc("TRN2", target_bir_lowering=False, debug=False,
                   num_devices=N_CORES)

    d = {}
    d["xqT"] = nc.dram_tensor("xqT", [F, S], BF16, kind="ExternalInput").ap()
    d["xkvT"] = nc.dram_tensor("xkvT", [F, S], BF16, kind="ExternalInput").ap()
    d["wqkv"] = nc.dram_tensor("wqkv", [F, 3 * 128], BF16, kind="ExternalInput").ap()
    d["b3"] = nc.dram_tensor("b3", [128, 3], F32, kind="ExternalInput").ap()
    d["tbl"] = nc.dram_tensor("tbl", [128, 512], BF16, kind="ExternalInput").ap()
    d["ident"] = nc.dram_tensor("ident", [128, 128], BF16, kind="ExternalInput").ap()
    xall_ap = nc.dram_tensor("xall", [HD + 1, S], BF16, kind="ExternalOutput").ap()
    xg_ap = (nc.dram_tensor("xg", [HD + 1, ng], BF16, kind="ExternalOutput").ap()
             if ng > 0 else None)

    SC = 512            # projection s-chunk (== PT)
    NSC = S // SC       # 8
    FT = F // 128       # 4 f-chunks
    kpg = max(1, 512 // max(ng, 1))          # B-logit ktiles per psum group
    nbg = (NKT + kpg - 1) // kpg if ng else 0

    Exp = mybir.ActivationFunctionType.Exp

    with tile.TileContext(nc) as tc:
        with (
            tc.tile_pool(name="const", bufs=1) as constp,
            tc.tile_pool(name="big", bufs=1) as bigp,
        ):
            # ---- constants: critical ones early on sync, the rest on gpsimd ----
            wqkv_sb = constp.tile([128, FT, 3 * 128], BF16, tag="wqkv")
            b3_sb = constp.tile([128, 3], F32, tag="b3")
            tbl_sb = constp.tile([128, 512], BF16, tag="tbl")
            nc.gpsimd.dma_start(tbl_sb[:], d["tbl"][:])
            id_sb = constp.tile([128, 128], BF16, tag="id")
            nc.gpsimd.dma_start(id_sb[:], d["ident"][:])

            # ---- full-input prefetch: sync->xq chunks, scalar->xkv chunks ----
            xq_sb = bigp.tile([128, FT, S], BF16, tag="xq")
            xkv_sb = bigp.tile([128, FT, S], BF16, tag="xkv")
            xqr = d["xqT"].rearrange("(c p) s -> p c s", p=128)
            xkvr = d["xkvT"].rearrange("(c p) s -> p c s", p=128)
            nc.sync.dma_start(xq_sb[:, :, 0:SC], xqr[:, :, 0:SC])
            nc.sync.dma_start(wqkv_sb[:],
                              d["wqkv"].rearrange("(c p) e -> p c e", p=128))
            nc.sync.dma_start(b3_sb[:], d["b3"][:])
            for u in range(1, NSC):
                cs = u * SC
                nc.sync.dma_start(xq_sb[:, :, cs:cs + SC], xqr[:, :, cs:cs + SC])
            for u in range(NSC):
                cs = u * SC
                nc.scalar.dma_start(xkv_sb[:, :, cs:cs + SC],
                                    xkvr[:, :, cs:cs + SC])

            qT = bigp.tile([128, S], BF16, tag="qT")     # rows 0:64 sw, 64:128 g
            kT = bigp.tile([128, S], BF16, tag="kT")
            # zero-padded variants for nominally-128-contract attention matmuls
            kTz = bigp.tile([128, S], BF16, tag="kTz")   # rows 0:64 k_sw, 64:128 zero
            nc.vector.memset(kTz[64:128, :], 0.0)
            if ng > 0:
                qTgz = bigp.tile([128, 128], BF16, tag="qTgz")  # rows 64:128 q_g
                nc.gpsimd.memset(qTgz[0:64, :], 0.0)
            # v natural: [s%128, ktile, {sw,g}, d|ones]
            vcomb = bigp.tile([128, NKT, 2, HD + 1], BF16, tag="vcomb")
            nc.vector.memset(vcomb[:, :, :, HD], 1.0)
            # unnormalized attention outputs (transposed) + denominators row 64
            xTall = bigp.tile([HD + 1, NPT, PT], BF16, tag="xTall")
            # global-key exp weights per pair (1..7), rows ng:128 kept zero
            if ng > 0:
                Egbuf = bigp.tile([128, NPT - 1, PT], BF16, tag="Egbuf")
                nc.gpsimd.memset(Egbuf[ng:128, :, :], 0.0)
                egB = bigp.tile([128, NKT, ng], BF16, tag="egB")
                xgB = bigp.tile([HD + 1, ng], BF16, tag="xgB")

            # ============ Phase A: projections + global-attention prep ==========
            with (
                tc.tile_pool(name="vtmp", bufs=2) as vtmpp,
                tc.tile_pool(name="pa", bufs=4, space="PSUM") as pap,
                tc.tile_pool(name="ptr", bufs=3, space="PSUM") as ptrp,
                tc.tile_pool(name="pbx", bufs=1, space="PSUM") as pbxp,
            ):
                def a_proj(sc):
                    ss = sc * SC
                    pq = pap.tile([128, SC], F32, tag="pa")
                    for ft in range(FT):
                        nc.tensor.matmul(pq[:], wqkv_sb[:, ft, 0:128],
                                         xq_sb[:, ft, ss:ss + SC],
                                         start=(ft == 0), stop=(ft == FT - 1))
                    nc.vector.tensor_scalar_add(qT[:, ss:ss + SC], pq[:],
                                                b3_sb[:, 0:1])
                    pk = pap.tile([128, SC], F32, tag="pa")
                    for ft in range(FT):
                        nc.tensor.matmul(pk[:], wqkv_sb[:, ft, 128:256],
                                         xkv_sb[:, ft, ss:ss + SC],
                                         start=(ft == 0), stop=(ft == FT - 1))
                    nc.vector.tensor_scalar_add(kT[:, ss:ss + SC], pk[:],
                                                b3_sb[:, 1:2])
                    nc.vector.tensor_scalar_add(kTz[0:64, ss:ss + SC],
                                                pk[0:64, :], b3_sb[0:64, 1:2])
                    if ng > 0 and sc == 0:
                        nc.vector.tensor_scalar_add(qTgz[64:128, 0:ng],
                                                    pq[64:128, 0:ng],
                                                    b3_sb[64:128, 0:1])
                    pv = pap.tile([128, SC], F32, tag="pa")
                    for ft in range(FT):
                        nc.tensor.matmul(pv[:], wqkv_sb[:, ft, 256:384],
                                         xkv_sb[:, ft, ss:ss + SC],
                                         start=(ft == 0), stop=(ft == FT - 1))
                    vt = vtmpp.tile([128, SC], BF16, tag="vt")
                    nc.vector.tensor_scalar_add(vt[:], pv[:], b3_sb[:, 2:3])
                    return vt

                def a_transpose(sc, vt):
                    for sb in range(SC // 128):
                        kt_idx = sc * (SC // 128) + sb
                        ptr = ptrp.tile([128, 128], BF16, tag="tr")
                        nc.tensor.transpose(ptr[:], vt[:, sb * 128:(sb + 1) * 128],
                                            id_sb[:])
                        src = ptr[:].rearrange("p (b x) -> p b x", b=2)
                        dst = vcomb[:, kt_idx, :, 0:HD]
                        if kt_idx % 2 == 0:
                            nc.vector.tensor_copy(dst, src)
                        else:
                            nc.scalar.copy(dst, src)

                def a_eg(T):
                    # global-key logits+exp for pair T (needs q chunk T only)
                    qs = T * PT
                    plg = pap.tile([ng, PT], F32, tag="pa")
                    nc.tensor.matmul(plg[:], kTz[:, 0:ng], qT[:, qs:qs + PT],
                                     start=True, stop=True)
                    nc.scalar.activation(Egbuf[0:ng, T - 1, :], plg[:],
                                         Exp, scale=0.125)

                def a_bgroup(g):
                    # global-query logits+exp for ktiles [g*kpg, (g+1)*kpg)
                    nkt_g = min(kpg, NKT - g * kpg)
                    plB = pap.tile([128, nkt_g * ng], F32, tag="pa")
                    for i in range(nkt_g):
                        kt = g * kpg + i
                        nc.tensor.matmul(plB[:, i * ng:(i + 1) * ng],
                                         kT[:, kt * KT:(kt + 1) * KT],
                                         qTgz[:, 0:ng],
                                         start=True, stop=True)
                    nc.scalar.activation(
                        egB[:, g * kpg:g * kpg + nkt_g, :],
                        plB[:].rearrange("p (a b) -> p a b", a=nkt_g),
                        Exp, scale=0.125)

                bstate = {}

                def b_av_chunk(c):
                    # AV over ktiles [8c, 8c+8): needs egB group c (after
                    # sc=2c+1) and vcomb ktiles (transposed after sc=2c+1)
                    if c == 0:
                        pxg = pbxp.tile([HD + 1, ng], F32, tag="BX")
                        bstate["pxg"] = pxg
                    pxg = bstate["pxg"]
                    for kt in range(8 * c, 8 * c + 8):
                        nc.tensor.matmul(pxg[:], vcomb[:, kt, 1, :],
                                         egB[:, kt, :],
                                         start=(kt == 0), stop=(kt == NKT - 1),
                                         skip_group_check=True)

                prev = None
                for sc in range(NSC):
                    vt = a_proj(sc)
                    if prev is not None:
                        a_transpose(sc - 1, prev)
                    prev = vt
                    if ng:
                        if sc >= 1:
                            a_eg(sc)
                        if sc % 2 == 1 and (sc - 1) // 2 < nbg:
                            a_bgroup((sc - 1) // 2)
                        if sc in (5, 7):
                            b_av_chunk(sc - 5 if sc == 5 else 1)
                a_transpose(NSC - 1, prev)
                if ng:
                    for g in range(4, nbg):
                        a_bgroup(g)
                    b_av_chunk(2)
                    b_av_chunk(3)
                    nc.vector.tensor_copy(xgB[:], bstate["pxg"][:])
                    nc.sync.dma_start(xg_ap[:], xgB[:])

            # ============ Phase C: paired sliding-window attention ==============
            with (
                tc.tile_pool(name="E", bufs=2) as ep,
                tc.tile_pool(name="pL", bufs=2, space="PSUM") as pLp,
                tc.tile_pool(name="pX", bufs=2, space="PSUM") as pXp,
            ):
                stash = {}

                def stage_L(T):
                    qs = T * PT
                    slots = _pair_slots(T, ng)
                    groups, emap = _pack_pair(T, ng)
                    E = ep.tile([128, 2560], BF16, tag="E")
                    for (placements, gw, ebase) in groups:
                        pl = pLp.tile([128, gw], F32, tag="L")
                        for ((s, j, w, qoff), pos) in placements:
                            nc.tensor.matmul(pl[:, pos:pos + w],
                                             kTz[:, j * KT:(j + 1) * KT],
                                             qT[:, qs + qoff:qs + qoff + w],
                                             start=True, stop=True)
                        nc.scalar.activation(E[:, ebase:ebase + gw], pl[:],
                                             Exp, scale=0.125)
                    # triangle masks (split between gpsimd and DVE)
                    for mi, (ecol, w, toff) in enumerate(_mask_ops(T, slots,
                                                                  emap, ng)):
                        eng = nc.gpsimd if mi % 2 == 0 else nc.vector
                        eng.tensor_mul(E[:, ecol:ecol + w],
                                       E[:, ecol:ecol + w],
                                       tbl_sb[:, toff:toff + w])
                    stash[T] = (E, emap)

                def stage_AV(T):
                    E, emap = stash.pop(T)
                    slots = _pair_slots(T, ng)
                    # s4 goes first: full [0,512) range (its start=True covers
                    # the whole psum tile) and its mask is ready earliest
                    ordered = ([sl for sl in slots if sl[0] == 4] +
                               [sl for sl in slots if sl[0] != 4])
                    has_g = ng > 0 and T >= 1
                    px2 = pXp.tile([HD + 1, PT], F32, tag="X")
                    n = len(ordered)
                    for idx, (s, j, w, qoff) in enumerate(ordered):
                        e = emap[s]
                        nc.tensor.matmul(px2[:, qoff:qoff + w],
                                         vcomb[:, j, 0, :], E[:, e:e + w],
                                         start=(idx == 0),
                                         stop=(idx == n - 1 and not has_g),
                                         skip_group_check=True)
                    if has_g:
                        nc.tensor.matmul(px2[:], vcomb[:, 0, 0, :],
                                         Egbuf[:, T - 1, :],
                                         start=False, stop=True,
                                         skip_group_check=True)
                    nc.vector.tensor_copy(xTall[:, T, :], px2[:])
                    nc.sync.dma_start(xall_ap[:, T * PT:(T + 1) * PT],
                                      xTall[:, T, :])

                for step in range(NPT + 1):
                    if step < NPT:
                        stage_L(step)
                    if step >= 1:
                        stage_AV(step - 1)

    nc.compile()
    return nc


@functools.lru_cache(maxsize=4)
def _get_program(ng: int):
    return _build_program(ng)


def kernel(inputs_q, inputs_kv, global_mask,
           w_q_sw, b_q_sw, w_k_sw, b_k_sw, w_v_sw, b_v_sw,
           w_q_g, b_q_g, w_k_g, b_k_g, w_v_g, b_v_g,
           w_out, b_out,
           _trace=False, _tmpdir=None):
    gm = np.asarray(global_mask[0]).astype(bool)
    ng = int(gm.sum())
    assert gm[:ng].all() and not gm[ng:].any(), "global_mask must be a prefix mask"
    assert ng <= 128, "kernel specialized for ng <= 128"

    xqT = np.ascontiguousarray(np.asarray(inputs_q[0], np.float32).T).astype(bfloat16)
    xkvT = np.ascontiguousarray(np.asarray(inputs_kv[0], np.float32).T).astype(bfloat16)
    tbl = _build_tbl(ng)
    ident = np.eye(128, dtype=bfloat16)

    nc = _get_program(ng)

    in_maps = []
    for h in range(N_CORES):
        wq = np.concatenate([w_q_sw[:, h, :], w_q_g[:, h, :]], axis=1)
        wk = np.concatenate([w_k_sw[:, h, :], w_k_g[:, h, :]], axis=1)
        wv = np.concatenate([w_v_sw[:, h, :], w_v_g[:, h, :]], axis=1)
        wqkv = np.concatenate([wq, wk, wv], axis=1).astype(bfloat16)
        b3 = np.stack([np.concatenate([b_q_sw[h], b_q_g[h]]).reshape(-1),
                       np.concatenate([b_k_sw[h], b_k_g[h]]).reshape(-1),
                       np.concatenate([b_v_sw[h], b_v_g[h]]).reshape(-1)],
                      axis=1).astype(np.float32)
        in_maps.append({
            "xqT": xqT, "xkvT": xkvT,
            "wqkv": wqkv, "b3": b3,
            "tbl": tbl, "ident": ident,
        })

    res = run_bass_kernel_spmd(nc, in_maps, list(range(N_CORES)),
                               trace=_trace, tmpdir=_tmpdir)
    out = np.zeros((S, F), np.float32)
    for h in range(N_CORES):
        xall = np.asarray(res.results[h]["xall"], dtype=np.float32)  # [65, S]
        xh = (xall[:HD] / xall[HD]).T                                # [S, 64]
        if ng > 0:
            xg = np.asarray(res.results[h]["xg"], dtype=np.float32)  # [65, ng]
            xh[:ng] = (xg[:HD] / xg[HD]).T
        out += xh @ np.asarray(w_out[h], np.float32)
    out += np.asarray(b_out, np.float32)
    if _trace:
        kernel._last_results = res
    return out[None].astype(np.float32)


# revision 53
# speedup vs baseline: 1.0193x; 1.0123x over previous
"""Longformer attention (B=1, S=4096, D=512, H=8, HD=64, window=512, nglobal=64)
on 8 Trainium2 NeuronCores, head-parallel (core c computes head c).

v6 layout strategy (per core):
  - All matmul operands bf16 (psum accumulation fp32). Host pre-transposes
    inputs to xT [512, 4096] bf16 and packs wq|wk|wv, bq|bk|bv, mask tables.
  - DMA issue parallelism: sync issues xq chunk 0 then the projection
    weights then the remaining xq chunks; scalar issues the xkv chunks;
    gpsimd issues the small late-use constants.
  - Projections transposed: qT/kT [128(d_sw|d_g), 4096]; v transposed to
    natural layout vcomb [s%128, kt, {sw,g}, d|1] via PE transposes one
    chunk behind. kTz/qTgz are zero-padded copies so every attention matmul
    is nominally 128-contract (keeps the PE HAM clock gate warm; padding
    rows multiply against zeros).
  - Global-query/global-key exp work (Eg per pair, B logit groups) runs
    during phase A: pair T's Eg needs only q-chunk T, B group g needs only
    k-chunks <= 2g+1. This leaves phase C's ACT with sliding-window exps
    only. B's AV accumulation runs at the end of phase A.
  - Sliding-window attention over 512-query supertile PAIRS (8 pairs),
    two-stage pipeline logits(T) | AV(T-1). Slots are packed into 2-bank
    psum groups with 512-col bank-aligned subslots (a matmul output must
    not cross a psum bank). Triangle masks are 128x128 table multiplies on
    gpsimd. AV starts with the full-width slot 4 so start=True covers the
    whole psum tile.
  - No on-device out-projection or normalization: the device exports
    xTall [65, 4096] (unnormalized AV outputs + ones-row denominators) and
    the host computes sum_h (x_h/den_h) @ wo_h + b_out.
"""
import os
import sys
import functools

for _p in ("/opt/trn_rl_repo",):
    if os.path.isdir(_p) and _p not in sys.path:
        sys.path.insert(0, _p)

import numpy as np
from ml_dtypes import bfloat16

import concourse.bass as bass
import concourse.tile as tile
from concourse import bacc, mybir
from concourse.bass_utils import run_bass_kernel_spmd

S = 4096
F = 512          # d_model
HD = 64          # head dim
H = 8
WIN = 512        # sliding window (left 256, right 256)
PT = 512         # query supertile pair
NPT = S // PT    # 8
KT = 128         # key tile
NKT = S // KT    # 32
N_CORES = 8
F32 = mybir.dt.float32
BF16 = mybir.dt.bfloat16

# per-slot stored query ranges (slot s covers ktile 4T-2+s, k_rel=128(s-2)+kk)
_SLOT_W = {0: 128, 1: 256, 2: 384, 3: 512, 4: 512, 5: 384, 6: 256, 7: 128}
_SLOT_QOFF = {0: 0, 1: 0, 2: 0, 3: 0, 4: 0, 5: 128, 6: 256, 7: 384}
# mask table column offsets
_TBL_A = 0      # qq' <= kk-1   (upper band edge)
_TBL_B = 128    # qq' >= kk     (lower band edge)
_TBL_G2 = 256   # A | (kk < ng)
_TBL_GO = 384   # kk < ng only


def _build_tbl(ng: int):
    kk = np.arange(KT)[:, None]
    qq = np.arange(KT)[None, :]
    A = (qq <= kk - 1).astype(np.float32)
    B = (qq >= kk).astype(np.float32)
    G2 = np.maximum(A, (kk < ng).astype(np.float32) * np.ones_like(A))
    GO = ((kk < ng).astype(np.float32) * np.ones_like(A))
    return np.concatenate([A, B, G2, GO], axis=1).astype(bfloat16)  # [128, 512]


def _pair_slots(T: int, ng: int):
    """[(s, ktile, width, qoff)] for pair T."""
    s_lo = 2 if T == 0 else 0
    s_hi = 6 if T == NPT - 1 else 8
    out = []
    for s in range(s_lo, s_hi):
        j = 4 * T - 2 + s
        w = _SLOT_W[s]
        if T == 0 and s == 2 and ng > 0:
            w = 512          # extend ktile-0 range so all queries see global keys
        out.append((s, j, w, _SLOT_QOFF[s]))
    return out


def _pack_pair(T: int, ng: int):
    """Pack slots into 2-bank psum groups with 512-col bank-aligned subslots
    (a matmul output may not cross a psum bank boundary).

    Returns (groups, emap): groups = [(placements, width, ebase)] with
    placements = [(slot, pos)]; emap[s] = column of slot s in the E tile.
    """
    slots = _pair_slots(T, ng)
    rem = sorted(slots, key=lambda x: -x[2])
    subslots = []
    while rem:
        big = rem.pop(0)
        sub, fill = [(big, 0)], big[2]
        while fill < 512:
            nxt = next((x for x in rem if x[2] <= 512 - fill), None)
            if nxt is None:
                break
            rem.remove(nxt)
            sub.append((nxt, fill))
            fill += nxt[2]
        subslots.append((sub, fill))
    groups, emap, ebase = [], {}, 0
    for i in range(0, len(subslots), 2):
        chunk = subslots[i:i + 2]
        placements, width = [], 0
        for gi, (sub, fill) in enumerate(chunk):
            base = gi * 512
            for (sl, off) in sub:
                assert off + sl[2] <= 512
                placements.append((sl, base + off))
                emap[sl[0]] = ebase + base + off
            width = base + fill
        groups.append((placements, width, ebase))
        ebase += width
    return groups, emap


def _mask_ops(T: int, slots, emap, ng: int):
    """[(ecol, width, tbl_off)] mask multiplies into the E tile."""
    ops = []
    for (s, j, w, qoff) in slots:
        e = emap[s]
        if s == 0:
            ops.append((e + 0, 128, _TBL_A))
        elif s == 1:
            ops.append((e + 128, 128, _TBL_A))
        elif s == 2:
            if T == 0 and ng > 0:
                ops.append((e + 256, 128, _TBL_G2))
                ops.append((e + 384, 128, _TBL_GO))
            else:
                ops.append((e + 256, 128, _TBL_A))
        elif s == 3:
            ops.append((e + 384, 128, _TBL_A))
        else:  # 4, 5, 6, 7
            ops.append((e + 0, 128, _TBL_B))
    return ops


def _build_program(ng: int):
    nc = bacc.Bacc("TRN2", target_bir_lowering=False, debug=False,
                   num_devices=N_CORES)

    d = {}
    d["xqT"] = nc.dram_tensor("xqT", [F, S], BF16, kind="ExternalInput").ap()
    d["xkvT"] = nc.dram_tensor("xkvT", [F, S], BF16, kind="ExternalInput").ap()
    d["wqkv"] = nc.dram_tensor("wqkv", [F, 3 * 128], BF16, kind="ExternalInput").ap()
    d["b3"] = nc.dram_tensor("b3", [128, 3], F32, kind="ExternalInput").ap()
    d["tbl"] = nc.dram_tensor("tbl", [128, 512], BF16, kind="ExternalInput").ap()
    d["ident"] = nc.dram_tensor("ident", [128, 128], BF16, kind="ExternalInput").ap()
    xall_ap = nc.dram_tensor("xall", [HD + 1, S], BF16, kind="ExternalOutput").ap()
    xg_ap = (nc.dram_tensor("xg", [HD + 1, ng], BF16, kind="ExternalOutput").ap()
             if ng > 0 else None)

    SC = 512            # projection s-chunk (== PT)
    NSC = S // SC       # 8
    FT = F // 128       # 4 f-chunks
    kpg = max(1, 512 // max(ng, 1))          # B-logit ktiles per psum group
    nbg = (NKT + kpg - 1) // kpg if ng else 0

    Exp = mybir.ActivationFunctionType.Exp

    with tile.TileContext(nc) as tc:
        with (
            tc.tile_pool(name="const", bufs=1) as constp,
            tc.tile_pool(name="big", bufs=1) as bigp,
        ):
            # ---- constants: critical ones early on sync, the rest on gpsimd ----
            wqkv_sb = constp.tile([128, FT, 3 * 128], BF16, tag="wqkv")
            b3_sb = constp.tile([128, 3], F32, tag="b3")
            tbl_sb = constp.tile([128, 512], BF16, tag="tbl")
            nc.gpsimd.dma_start(tbl_sb[:], d["tbl"][:])
            id_sb = constp.tile([128, 128], BF16, tag="id")
            nc.gpsimd.dma_start(id_sb[:], d["ident"][:])

            # ---- full-input prefetch: sync->xq chunks, scalar->xkv chunks ----
            xq_sb = bigp.tile([128, FT, S], BF16, tag="xq")
            xkv_sb = bigp.tile([128, FT, S], BF16, tag="xkv")
            xqr = d["xqT"].rearrange("(c p) s -> p c s", p=128)
            xkvr = d["xkvT"].rearrange("(c p) s -> p c s", p=128)
            nc.sync.dma_start(xq_sb[:, :, 0:SC], xqr[:, :, 0:SC])
            nc.sync.dma_start(wqkv_sb[:],
                              d["wqkv"].rearrange("(c p) e -> p c e", p=128))
            nc.sync.dma_start(b3_sb[:], d["b3"][:])
            for u in range(1, NSC):
                cs = u * SC
                nc.sync.dma_start(xq_sb[:, :, cs:cs + SC], xqr[:, :, cs:cs + SC])
            for u in range(NSC):
                cs = u * SC
                nc.scalar.dma_start(xkv_sb[:, :, cs:cs + SC],
                                    xkvr[:, :, cs:cs + SC])

            qT = bigp.tile([128, S], BF16, tag="qT")     # rows 0:64 sw, 64:128 g
            kT = bigp.tile([128, S], BF16, tag="kT")
            # zero-padded variants for nominally-128-contract attention matmuls
            kTz = bigp.tile([128, S], BF16, tag="kTz")   # rows 0:64 k_sw, 64:128 zero
            nc.vector.memset(kTz[64:128, :], 0.0)
            if ng > 0:
                qTgz = bigp.tile([128, 128], BF16, tag="qTgz")  # rows 64:128 q_g
                nc.gpsimd.memset(qTgz[0:64, :], 0.0)
            # v natural: [s%128, ktile, {sw,g}, d|ones]
            vcomb = bigp.tile([128, NKT, 2, HD + 1], BF16, tag="vcomb")
            nc.vector.memset(vcomb[:, :, :, HD], 1.0)
            # unnormalized attention outputs (transposed) + denominators row 64
            xTall = bigp.tile([HD + 1, NPT, PT], BF16, tag="xTall")
            # global-key exp weights per pair (1..7), rows ng:128 kept zero
            if ng > 0:
                Egbuf = bigp.tile([128, NPT - 1, PT], BF16, tag="Egbuf")
                nc.gpsimd.memset(Egbuf[ng:128, :, :], 0.0)
                egB = bigp.tile([128, NKT, ng], BF16, tag="egB")
                xgB = bigp.tile([HD + 1, ng], BF16, tag="xgB")

            # ============ Phase A: projections + global-attention prep ==========
            with (
                tc.tile_pool(name="vtmp", bufs=2) as vtmpp,
                tc.tile_pool(name="pa", bufs=4, space="PSUM") as pap,
                tc.tile_pool(name="ptr", bufs=3, space="PSUM") as ptrp,
                tc.tile_pool(name="pbx", bufs=1, space="PSUM") as pbxp,
            ):
                def a_proj(sc):
                    ss = sc * SC
                    pq = pap.tile([128, SC], F32, tag="pa")
                    for ft in range(FT):
                        nc.tensor.matmul(pq[:], wqkv_sb[:, ft, 0:128],
                                         xq_sb[:, ft, ss:ss + SC],
                                         start=(ft == 0), stop=(ft == FT - 1))
                    nc.vector.tensor_scalar_add(qT[:, ss:ss + SC], pq[:],
                                                b3_sb[:, 0:1])
                    pk = pap.tile([128, SC], F32, tag="pa")
                    for ft in range(FT):
                        nc.tensor.matmul(pk[:], wqkv_sb[:, ft, 128:256],
                                         xkv_sb[:, ft, ss:ss + SC],
                                         start=(ft == 0), stop=(ft == FT - 1))
                    nc.vector.tensor_scalar_add(kT[:, ss:ss + SC], pk[:],
                                                b3_sb[:, 1:2])
                    nc.vector.tensor_scalar_add(kTz[0:64, ss:ss + SC],
                                                pk[0:64, :], b3_sb[0:64, 1:2])
                    if ng > 0 and sc == 0:
                        nc.vector.tensor_scalar_add(qTgz[64:128, 0:ng],
                                                    pq[64:128, 0:ng],
                                                    b3_sb[64:128, 0:1])
                    pv = pap.tile([128, SC], F32, tag="pa")
                    for ft in range(FT):
                        nc.tensor.matmul(pv[:], wqkv_sb[:, ft, 256:384],
                                         xkv_sb[:, ft, ss:ss + SC],
                                         start=(ft == 0), stop=(ft == FT - 1))
                    vt = vtmpp.tile([128, SC], BF16, tag="vt")
                    nc.vector.tensor_scalar_add(vt[:], pv[:], b3_sb[:, 2:3])
                    return vt

                def a_transpose(sc, vt):
                    for sb in range(SC // 128):
                        kt_idx = sc * (SC // 128) + sb
                        ptr = ptrp.tile([128, 128], BF16, tag="tr")
                        nc.tensor.transpose(ptr[:], vt[:, sb * 128:(sb + 1) * 128],
                                            id_sb[:])
                        src = ptr[:].rearrange("p (b x) -> p b x", b=2)
                        dst = vcomb[:, kt_idx, :, 0:HD]
                        if kt_idx % 2 == 0:
                            nc.vector.tensor_copy(dst, src)
                        else:
                            nc.scalar.copy(dst, src)

                def a_eg(T):
                    # global-key logits+exp for pair T (needs q chunk T only)
                    qs = T * PT
                    plg = pap.tile([ng, PT], F32, tag="pa")
                    nc.tensor.matmul(plg[:], kTz[:, 0:ng], qT[:, qs:qs + PT],
                                     start=True, stop=True)
                    nc.scalar.activation(Egbuf[0:ng, T - 1, :], plg[:],
                                         Exp, scale=0.125)

                def a_bgroup(g):
                    # global-query logits+exp for ktiles [g*kpg, (g+1)*kpg)
                    nkt_g = min(kpg, NKT - g * kpg)
                    plB = pap.tile([128, nkt_g * ng], F32, tag="pa")
                    for i in range(nkt_g):
                        kt = g * kpg + i
                        nc.tensor.matmul(plB[:, i * ng:(i + 1) * ng],
                                         kT[:, kt * KT:(kt + 1) * KT],
                                         qTgz[:, 0:ng],
                                         start=True, stop=True)
                    nc.scalar.activation(
                        egB[:, g * kpg:g * kpg + nkt_g, :],
                        plB[:].rearrange("p (a b) -> p a b", a=nkt_g),
                        Exp, scale=0.125)

                bstate = {}

                def b_av_chunk(c):
                    # AV over ktiles [8c, 8c+8): needs egB group c (after
                    # sc=2c+1) and vcomb ktiles (transposed after sc=2c+1)
                    if c == 0:
                        pxg = pbxp.tile([HD + 1, ng], F32, tag="BX")
                        bstate["pxg"] = pxg
                    pxg = bstate["pxg"]
                    for kt in range(8 * c, 8 * c + 8):
                        nc.tensor.matmul(pxg[:], vcomb[:, kt, 1, :],
                                         egB[:, kt, :],
                                         start=(kt == 0), stop=(kt == NKT - 1),
                                         skip_group_check=True)

                prev = None
                for sc in range(NSC):
                    vt = a_proj(sc)
                    if prev is not None:
                        a_transpose(sc - 1, prev)
                    prev = vt
                    if ng:
                        if sc >= 1:
                            a_eg(sc)
                        if sc % 2 == 1 and (sc - 1) // 2 < nbg:
                            a_bgroup((sc - 1) // 2)
                        if sc in (5, 7):
                            b_av_chunk(sc - 5 if sc == 5 else 1)
                a_transpose(NSC - 1, prev)
                if ng:
                    for g in range(4, nbg):
                        a_bgroup(g)
                    b_av_chunk(2)
                    b_av_chunk(3)
                    nc.vector.tensor_copy(xgB[:], bstate["pxg"][:])
                    nc.sync.dma_start(xg_ap[:], xgB[:])

            # ============ Phase C: paired sliding-window attention ==============
            with (
                tc.tile_pool(name="E", bufs=2) as ep,
                tc.tile_pool(name="pL", bufs=3, space="PSUM") as pLp,
                tc.tile_pool(name="pX", bufs=2, space="PSUM") as pXp,
            ):
                stash = {}

                def stage_L(T):
                    qs = T * PT
                    slots = _pair_slots(T, ng)
                    groups, emap = _pack_pair(T, ng)
                    E = ep.tile([128, 2560], BF16, tag="E")
                    for (placements, gw, ebase) in groups:
                        pl = pLp.tile([128, gw], F32, tag="L")
                        for ((s, j, w, qoff), pos) in placements:
                            nc.tensor.matmul(pl[:, pos:pos + w],
                                             kTz[:, j * KT:(j + 1) * KT],
                                             qT[:, qs + qoff:qs + qoff + w],
                                             start=True, stop=True)
                        nc.scalar.activation(E[:, ebase:ebase + gw], pl[:],
                                             Exp, scale=0.125)
                    # triangle masks (split between gpsimd and DVE)
                    for mi, (ecol, w, toff) in enumerate(_mask_ops(T, slots,
                                                                  emap, ng)):
                        eng = nc.gpsimd if mi % 2 == 0 else nc.vector
                        eng.tensor_mul(E[:, ecol:ecol + w],
                                       E[:, ecol:ecol + w],
                                       tbl_sb[:, toff:toff + w])
                    stash[T] = (E, emap)

                def stage_AV(T):
                    E, emap = stash.pop(T)
                    slots = _pair_slots(T, ng)
                    # s4 goes first: full [0,512) range (its start=True covers
                    # the whole psum tile) and its mask is ready earliest
                    ordered = ([sl for sl in slots if sl[0] == 4] +
                               [sl for sl in slots if sl[0] != 4])
                    has_g = ng > 0 and T >= 1
                    px2 = pXp.tile([HD + 1, PT], F32, tag="X")
                    n = len(ordered)
                    for idx, (s, j, w, qoff) in enumerate(ordered):
                        e = emap[s]
                        nc.tensor.matmul(px2[:, qoff:qoff + w],
                                         vcomb[:, j, 0, :], E[:, e:e + w],
                                         start=(idx == 0),
                                         stop=(idx == n - 1 and not has_g),
                                         skip_group_check=True)
                    if has_g:
                        nc.tensor.matmul(px2[:], vcomb[:, 0, 0, :],
                                         Egbuf[:, T - 1, :],
                                         start=False, stop=True,
                                         skip_group_check=True)
                    nc.vector.tensor_copy(xTall[:, T, :], px2[:])
                    nc.sync.dma_start(xall_ap[:, T * PT:(T + 1) * PT],
                                      xTall[:, T, :])

                for step in range(NPT + 1):
                    if step < NPT:
                        stage_L(step)
                    if step >= 1:
                        stage_AV(step - 1)

    nc.compile()
    return nc


@functools.lru_cache(maxsize=4)
def _get_program(ng: int):
    return _build_program(ng)


def kernel(inputs_q, inputs_kv, global_mask,
           w_q_sw, b_q_sw, w_k_sw, b_k_sw, w_v_sw, b_v_sw,
           w_q_g, b_q_g, w_k_g, b_k_g, w_v_g, b_v_g,
           w_out, b_out,
           _trace=False, _tmpdir=None):
    gm = np.asarray(global_mask[0]).astype(bool)
    ng = int(gm.sum())
    assert gm[:ng].all() and not gm[ng:].any(), "global_mask must be a prefix mask"
    assert ng <= 128, "kernel specialized for ng <= 128"

    xqT = np.ascontiguousarray(np.asarray(inputs_q[0], np.float32).T).astype(bfloat16)
    xkvT = np.ascontiguousarray(np.asarray(inputs_kv[0], np.float32).T).astype(bfloat16)
    tbl = _build_tbl(ng)
    ident = np.eye(128, dtype=bfloat16)

    nc = _get_program(ng)

    in_maps = []
    for h in range(N_CORES):
        wq = np.concatenate([w_q_sw[:, h, :], w_q_g[:, h, :]], axis=1)
        wk = np.concatenate([w_k_sw[:, h, :], w_k_g[:, h, :]], axis=1)
        wv = np.concatenate([w_v_sw[:, h, :], w_v_g[:, h, :]], axis=1)
        wqkv = np.concatenate([wq, wk, wv], axis=1).astype(bfloat16)
        b3 = np.stack([np.concatenate([b_q_sw[h], b_q_g[h]]).reshape(-1),
                       np.concatenate([b_k_sw[h], b_k_g[h]]).reshape(-1),
                       np.concatenate([b_v_sw[h], b_v_g[h]]).reshape(-1)],
                      axis=1).astype(np.float32)
        in_maps.append({
            "xqT": xqT, "xkvT": xkvT,
            "wqkv": wqkv, "b3": b3,
            "tbl": tbl, "ident": ident,
        })

    res = run_bass_kernel_spmd(nc, in_maps, list(range(N_CORES)),
                               trace=_trace, tmpdir=_tmpdir)
    out = np.zeros((S, F), np.float32)
    for h in range(N_CORES):
        xall = np.asarray(res.results[h]["xall"], dtype=np.float32)  # [65, S]
        xh = (xall[:HD] / xall[HD]).T                                # [S, 64]
        if ng > 0:
            xg = np.asarray(res.results[h]["xg"], dtype=np.float32)  # [65, ng]
            xh[:ng] = (xg[:HD] / xg[HD]).T
        out += xh @ np.asarray(w_out[h], np.float32)
    out += np.asarray(b_out, np.float32)
    if _trace:
        kernel._last_results = res
    return out[None].astype(np.float32)


# revision 57
# speedup vs baseline: 1.0262x; 1.0068x over previous
"""Longformer attention (B=1, S=4096, D=512, H=8, HD=64, window=512, nglobal=64)
on 8 Trainium2 NeuronCores, head-parallel (core c computes head c).

v6 layout strategy (per core):
  - All matmul operands bf16 (psum accumulation fp32). Host pre-transposes
    inputs to xT [512, 4096] bf16 and packs wq|wk|wv, bq|bk|bv, mask tables.
  - DMA issue parallelism: sync issues xq chunk 0 then the projection
    weights then the remaining xq chunks; scalar issues the xkv chunks;
    gpsimd issues the small late-use constants.
  - Projections transposed: qT/kT [128(d_sw|d_g), 4096]; v transposed to
    natural layout vcomb [s%128, kt, {sw,g}, d|1] via PE transposes one
    chunk behind. kTz/qTgz are zero-padded copies so every attention matmul
    is nominally 128-contract (keeps the PE HAM clock gate warm; padding
    rows multiply against zeros).
  - Global-query/global-key exp work (Eg per pair, B logit groups) runs
    during phase A: pair T's Eg needs only q-chunk T, B group g needs only
    k-chunks <= 2g+1. This leaves phase C's ACT with sliding-window exps
    only. B's AV accumulation runs at the end of phase A.
  - Sliding-window attention over 512-query supertile PAIRS (8 pairs),
    two-stage pipeline logits(T) | AV(T-1). Slots are packed into 2-bank
    psum groups with 512-col bank-aligned subslots (a matmul output must
    not cross a psum bank). Triangle masks are 128x128 table multiplies on
    gpsimd. AV starts with the full-width slot 4 so start=True covers the
    whole psum tile.
  - No on-device out-projection or normalization: the device exports
    xTall [65, 4096] (unnormalized AV outputs + ones-row denominators) and
    the host computes sum_h (x_h/den_h) @ wo_h + b_out.
"""
import os
import sys
import functools

for _p in ("/opt/trn_rl_repo",):
    if os.path.isdir(_p) and _p not in sys.path:
        sys.path.insert(0, _p)

import numpy as np
from ml_dtypes import bfloat16

import concourse.bass as bass
import concourse.tile as tile
from concourse import bacc, mybir
from concourse.bass_utils import run_bass_kernel_spmd

S = 4096
F = 512          # d_model
HD = 64          # head dim
H = 8
WIN = 512        # sliding window (left 256, right 256)
PT = 512         # query supertile pair
NPT = S // PT    # 8
KT = 128         # key tile
NKT = S // KT    # 32
N_CORES = 8
F32 = mybir.dt.float32
BF16 = mybir.dt.bfloat16

# per-slot stored query ranges (slot s covers ktile 4T-2+s, k_rel=128(s-2)+kk)
_SLOT_W = {0: 128, 1: 256, 2: 384, 3: 512, 4: 512, 5: 384, 6: 256, 7: 128}
_SLOT_QOFF = {0: 0, 1: 0, 2: 0, 3: 0, 4: 0, 5: 128, 6: 256, 7: 384}
# mask table column offsets
_TBL_A = 0      # qq' <= kk-1   (upper band edge)
_TBL_B = 128    # qq' >= kk     (lower band edge)
_TBL_G2 = 256   # A | (kk < ng)
_TBL_GO = 384   # kk < ng only


def _build_tbl(ng: int):
    kk = np.arange(KT)[:, None]
    qq = np.arange(KT)[None, :]
    A = (qq <= kk - 1).astype(np.float32)
    B = (qq >= kk).astype(np.float32)
    G2 = np.maximum(A, (kk < ng).astype(np.float32) * np.ones_like(A))
    GO = ((kk < ng).astype(np.float32) * np.ones_like(A))
    return np.concatenate([A, B, G2, GO], axis=1).astype(bfloat16)  # [128, 512]


def _pair_slots(T: int, ng: int):
    """[(s, ktile, width, qoff)] for pair T."""
    s_lo = 2 if T == 0 else 0
    s_hi = 6 if T == NPT - 1 else 8
    out = []
    for s in range(s_lo, s_hi):
        j = 4 * T - 2 + s
        w = _SLOT_W[s]
        if T == 0 and s == 2 and ng > 0:
            w = 512          # extend ktile-0 range so all queries see global keys
        out.append((s, j, w, _SLOT_QOFF[s]))
    return out


def _pack_pair(T: int, ng: int):
    """Pack slots into 2-bank psum groups with 512-col bank-aligned subslots
    (a matmul output may not cross a psum bank boundary).

    Returns (groups, emap): groups = [(placements, width, ebase)] with
    placements = [(slot, pos)]; emap[s] = column of slot s in the E tile.
    """
    slots = _pair_slots(T, ng)
    rem = sorted(slots, key=lambda x: -x[2])
    subslots = []
    while rem:
        big = rem.pop(0)
        sub, fill = [(big, 0)], big[2]
        while fill < 512:
            nxt = next((x for x in rem if x[2] <= 512 - fill), None)
            if nxt is None:
                break
            rem.remove(nxt)
            sub.append((nxt, fill))
            fill += nxt[2]
        subslots.append((sub, fill))
    groups, emap, ebase = [], {}, 0
    for i in range(0, len(subslots), 2):
        chunk = subslots[i:i + 2]
        placements, width = [], 0
        for gi, (sub, fill) in enumerate(chunk):
            base = gi * 512
            for (sl, off) in sub:
                assert off + sl[2] <= 512
                placements.append((sl, base + off))
                emap[sl[0]] = ebase + base + off
            width = base + fill
        groups.append((placements, width, ebase))
        ebase += width
    return groups, emap


def _mask_ops(T: int, slots, emap, ng: int):
    """[(ecol, width, tbl_off)] mask multiplies into the E tile."""
    ops = []
    for (s, j, w, qoff) in slots:
        e = emap[s]
        if s == 0:
            ops.append((e + 0, 128, _TBL_A))
        elif s == 1:
            ops.append((e + 128, 128, _TBL_A))
        elif s == 2:
            if T == 0 and ng > 0:
                ops.append((e + 256, 128, _TBL_G2))
                ops.append((e + 384, 128, _TBL_GO))
            else:
                ops.append((e + 256, 128, _TBL_A))
        elif s == 3:
            ops.append((e + 384, 128, _TBL_A))
        else:  # 4, 5, 6, 7
            ops.append((e + 0, 128, _TBL_B))
    return ops


def _build_program(ng: int):
    nc = bacc.Bacc("TRN2", target_bir_lowering=False, debug=False,
                   num_devices=N_CORES)

    d = {}
    d["xqT"] = nc.dram_tensor("xqT", [F, S], BF16, kind="ExternalInput").ap()
    d["xkvT"] = nc.dram_tensor("xkvT", [F, S], BF16, kind="ExternalInput").ap()
    d["wqkv"] = nc.dram_tensor("wqkv", [F, 3 * 128], BF16, kind="ExternalInput").ap()
    d["b3"] = nc.dram_tensor("b3", [128, 3], F32, kind="ExternalInput").ap()
    d["tbl"] = nc.dram_tensor("tbl", [128, 512], BF16, kind="ExternalInput").ap()
    d["ident"] = nc.dram_tensor("ident", [128, 128], BF16, kind="ExternalInput").ap()
    xall_ap = nc.dram_tensor("xall", [HD + 1, S], BF16, kind="ExternalOutput").ap()
    xg_ap = (nc.dram_tensor("xg", [HD + 1, ng], BF16, kind="ExternalOutput").ap()
             if ng > 0 else None)

    SC = 512            # projection s-chunk (== PT)
    NSC = S // SC       # 8
    FT = F // 128       # 4 f-chunks
    kpg = max(1, 512 // max(ng, 1))          # B-logit ktiles per psum group
    nbg = (NKT + kpg - 1) // kpg if ng else 0

    Exp = mybir.ActivationFunctionType.Exp

    with tile.TileContext(nc) as tc:
        with (
            tc.tile_pool(name="const", bufs=1) as constp,
            tc.tile_pool(name="big", bufs=1) as bigp,
        ):
            # ---- constants: critical ones early on sync, the rest on gpsimd ----
            wqkv_sb = constp.tile([128, FT, 3 * 128], BF16, tag="wqkv")
            b3_sb = constp.tile([128, 3], F32, tag="b3")
            tbl_sb = constp.tile([128, 512], BF16, tag="tbl")
            nc.gpsimd.dma_start(tbl_sb[:], d["tbl"][:])
            id_sb = constp.tile([128, 128], BF16, tag="id")
            nc.gpsimd.dma_start(id_sb[:], d["ident"][:])

            # ---- full-input prefetch: sync->xq chunks, scalar->xkv chunks ----
            xq_sb = bigp.tile([128, FT, S], BF16, tag="xq")
            xkv_sb = bigp.tile([128, FT, S], BF16, tag="xkv")
            xqr = d["xqT"].rearrange("(c p) s -> p c s", p=128)
            xkvr = d["xkvT"].rearrange("(c p) s -> p c s", p=128)
            nc.sync.dma_start(xq_sb[:, :, 0:SC], xqr[:, :, 0:SC])
            nc.sync.dma_start(wqkv_sb[:],
                              d["wqkv"].rearrange("(c p) e -> p c e", p=128))
            nc.sync.dma_start(b3_sb[:], d["b3"][:])
            for u in range(1, NSC):
                cs = u * SC
                nc.sync.dma_start(xq_sb[:, :, cs:cs + SC], xqr[:, :, cs:cs + SC])
            for u in range(NSC):
                cs = u * SC
                nc.scalar.dma_start(xkv_sb[:, :, cs:cs + SC],
                                    xkvr[:, :, cs:cs + SC])

            kT = bigp.tile([128, S], BF16, tag="kT")     # rows 0:64 sw, 64:128 g
            # zero-padded variants for nominally-128-contract attention matmuls
            qTz = bigp.tile([128, S], BF16, tag="qTz")   # rows 0:64 q_sw, 64:128 zero
            nc.vector.memset(qTz[64:128, :], 0.0)
            if ng > 0:
                qTgz = bigp.tile([128, 128], BF16, tag="qTgz")  # rows 64:128 q_g
                nc.gpsimd.memset(qTgz[0:64, :], 0.0)
            # v natural: [s%128, ktile, {sw,g}, d|ones]
            vcomb = bigp.tile([128, NKT, 2, HD + 1], BF16, tag="vcomb")
            nc.vector.memset(vcomb[:, :, :, HD], 1.0)
            # unnormalized attention outputs (transposed) + denominators row 64
            xTall = bigp.tile([HD + 1, NPT, PT], BF16, tag="xTall")
            # global-key exp weights per pair (1..7), rows ng:128 kept zero
            if ng > 0:
                Egbuf = bigp.tile([128, NPT - 1, PT], BF16, tag="Egbuf")
                nc.gpsimd.memset(Egbuf[ng:128, :, :], 0.0)
                egB = bigp.tile([128, NKT, ng], BF16, tag="egB")
                xgB = bigp.tile([HD + 1, ng], BF16, tag="xgB")

            # ============ Phase A: projections + global-attention prep ==========
            with (
                tc.tile_pool(name="vtmp", bufs=2) as vtmpp,
                tc.tile_pool(name="pa", bufs=4, space="PSUM") as pap,
                tc.tile_pool(name="ptr", bufs=3, space="PSUM") as ptrp,
                tc.tile_pool(name="pbx", bufs=1, space="PSUM") as pbxp,
            ):
                def a_proj(sc):
                    ss = sc * SC
                    pq = pap.tile([128, SC], F32, tag="pa")
                    for ft in range(FT):
                        nc.tensor.matmul(pq[:], wqkv_sb[:, ft, 0:128],
                                         xq_sb[:, ft, ss:ss + SC],
                                         start=(ft == 0), stop=(ft == FT - 1))
                    nc.vector.tensor_scalar_add(qTz[0:64, ss:ss + SC],
                                                pq[0:64, :], b3_sb[0:64, 0:1])
                    pk = pap.tile([128, SC], F32, tag="pa")
                    for ft in range(FT):
                        nc.tensor.matmul(pk[:], wqkv_sb[:, ft, 128:256],
                                         xkv_sb[:, ft, ss:ss + SC],
                                         start=(ft == 0), stop=(ft == FT - 1))
                    nc.vector.tensor_scalar_add(kT[:, ss:ss + SC], pk[:],
                                                b3_sb[:, 1:2])
                    if ng > 0 and sc == 0:
                        nc.vector.tensor_scalar_add(qTgz[64:128, 0:ng],
                                                    pq[64:128, 0:ng],
                                                    b3_sb[64:128, 0:1])
                    pv = pap.tile([128, SC], F32, tag="pa")
                    for ft in range(FT):
                        nc.tensor.matmul(pv[:], wqkv_sb[:, ft, 256:384],
                                         xkv_sb[:, ft, ss:ss + SC],
                                         start=(ft == 0), stop=(ft == FT - 1))
                    vt = vtmpp.tile([128, SC], BF16, tag="vt")
                    nc.vector.tensor_scalar_add(vt[:], pv[:], b3_sb[:, 2:3])
                    return vt

                def a_transpose(sc, vt):
                    for sb in range(SC // 128):
                        kt_idx = sc * (SC // 128) + sb
                        ptr = ptrp.tile([128, 128], BF16, tag="tr")
                        nc.tensor.transpose(ptr[:], vt[:, sb * 128:(sb + 1) * 128],
                                            id_sb[:])
                        src = ptr[:].rearrange("p (b x) -> p b x", b=2)
                        dst = vcomb[:, kt_idx, :, 0:HD]
                        if kt_idx % 2 == 0:
                            nc.vector.tensor_copy(dst, src)
                        else:
                            nc.scalar.copy(dst, src)

                def a_eg(T):
                    # global-key logits+exp for pair T (needs q chunk T only)
                    qs = T * PT
                    plg = pap.tile([ng, PT], F32, tag="pa")
                    nc.tensor.matmul(plg[:], kT[:, 0:ng], qTz[:, qs:qs + PT],
                                     start=True, stop=True)
                    nc.scalar.activation(Egbuf[0:ng, T - 1, :], plg[:],
                                         Exp, scale=0.125)

                def a_bgroup(g):
                    # global-query logits+exp for ktiles [g*kpg, (g+1)*kpg)
                    nkt_g = min(kpg, NKT - g * kpg)
                    plB = pap.tile([128, nkt_g * ng], F32, tag="pa")
                    for i in range(nkt_g):
                        kt = g * kpg + i
                        nc.tensor.matmul(plB[:, i * ng:(i + 1) * ng],
                                         kT[:, kt * KT:(kt + 1) * KT],
                                         qTgz[:, 0:ng],
                                         start=True, stop=True)
                    nc.scalar.activation(
                        egB[:, g * kpg:g * kpg + nkt_g, :],
                        plB[:].rearrange("p (a b) -> p a b", a=nkt_g),
                        Exp, scale=0.125)

                bstate = {}

                def b_av_chunk(c):
                    # AV over ktiles [8c, 8c+8): needs egB group c (after
                    # sc=2c+1) and vcomb ktiles (transposed after sc=2c+1)
                    if c == 0:
                        pxg = pbxp.tile([HD + 1, ng], F32, tag="BX")
                        bstate["pxg"] = pxg
                    pxg = bstate["pxg"]
                    for kt in range(8 * c, 8 * c + 8):
                        nc.tensor.matmul(pxg[:], vcomb[:, kt, 1, :],
                                         egB[:, kt, :],
                                         start=(kt == 0), stop=(kt == NKT - 1),
                                         skip_group_check=True)

                prev = None
                for sc in range(NSC):
                    vt = a_proj(sc)
                    if prev is not None:
                        a_transpose(sc - 1, prev)
                    prev = vt
                    if ng:
                        if sc >= 1:
                            a_eg(sc)
                        if sc % 2 == 1 and (sc - 1) // 2 < nbg:
                            a_bgroup((sc - 1) // 2)
                        if sc in (5, 7):
                            b_av_chunk(sc - 5 if sc == 5 else 1)
                a_transpose(NSC - 1, prev)
                if ng:
                    for g in range(4, nbg):
                        a_bgroup(g)
                    b_av_chunk(2)
                    b_av_chunk(3)
                    nc.vector.tensor_copy(xgB[:], bstate["pxg"][:])
                    nc.sync.dma_start(xg_ap[:], xgB[:])

            # ============ Phase C: paired sliding-window attention ==============
            with (
                tc.tile_pool(name="E", bufs=2) as ep,
                tc.tile_pool(name="pL", bufs=3, space="PSUM") as pLp,
                tc.tile_pool(name="pX", bufs=2, space="PSUM") as pXp,
            ):
                stash = {}

                def stage_L(T):
                    qs = T * PT
                    slots = _pair_slots(T, ng)
                    groups, emap = _pack_pair(T, ng)
                    E = ep.tile([128, 2560], BF16, tag="E")
                    for (placements, gw, ebase) in groups:
                        pl = pLp.tile([128, gw], F32, tag="L")
                        for ((s, j, w, qoff), pos) in placements:
                            nc.tensor.matmul(pl[:, pos:pos + w],
                                             kT[:, j * KT:(j + 1) * KT],
                                             qTz[:, qs + qoff:qs + qoff + w],
                                             start=True, stop=True)
                        nc.scalar.activation(E[:, ebase:ebase + gw], pl[:],
                                             Exp, scale=0.125)
                    # triangle masks (split between gpsimd and DVE)
                    for mi, (ecol, w, toff) in enumerate(_mask_ops(T, slots,
                                                                  emap, ng)):
                        eng = nc.gpsimd if mi % 2 == 0 else nc.vector
                        eng.tensor_mul(E[:, ecol:ecol + w],
                                       E[:, ecol:ecol + w],
                                       tbl_sb[:, toff:toff + w])
                    stash[T] = (E, emap)

                def stage_AV(T):
                    E, emap = stash.pop(T)
                    slots = _pair_slots(T, ng)
                    # s4 goes first: full [0,512) range (its start=True covers
                    # the whole psum tile) and its mask is ready earliest
                    ordered = ([sl for sl in slots if sl[0] == 4] +
                               [sl for sl in slots if sl[0] != 4])
                    has_g = ng > 0 and T >= 1
                    px2 = pXp.tile([HD + 1, PT], F32, tag="X")
                    n = len(ordered)
                    for idx, (s, j, w, qoff) in enumerate(ordered):
                        e = emap[s]
                        nc.tensor.matmul(px2[:, qoff:qoff + w],
                                         vcomb[:, j, 0, :], E[:, e:e + w],
                                         start=(idx == 0),
                                         stop=(idx == n - 1 and not has_g),
                                         skip_group_check=True)
                    if has_g:
                        nc.tensor.matmul(px2[:], vcomb[:, 0, 0, :],
                                         Egbuf[:, T - 1, :],
                                         start=False, stop=True,
                                         skip_group_check=True)
                    nc.vector.tensor_copy(xTall[:, T, :], px2[:])
                    nc.sync.dma_start(xall_ap[:, T * PT:(T + 1) * PT],
                                      xTall[:, T, :])

                for step in range(NPT + 1):
                    if step < NPT:
                        stage_L(step)
                    if step >= 1:
                        stage_AV(step - 1)

    nc.compile()
    return nc


@functools.lru_cache(maxsize=4)
def _get_program(ng: int):
    return _build_program(ng)


def kernel(inputs_q, inputs_kv, global_mask,
           w_q_sw, b_q_sw, w_k_sw, b_k_sw, w_v_sw, b_v_sw,
           w_q_g, b_q_g, w_k_g, b_k_g, w_v_g, b_v_g,
           w_out, b_out,
           _trace=False, _tmpdir=None):
    gm = np.asarray(global_mask[0]).astype(bool)
    ng = int(gm.sum())
    assert gm[:ng].all() and not gm[ng:].any(), "global_mask must be a prefix mask"
    assert ng <= 128, "kernel specialized for ng <= 128"

    xqT = np.ascontiguousarray(np.asarray(inputs_q[0], np.float32).T).astype(bfloat16)
    xkvT = np.ascontiguousarray(np.asarray(inputs_kv[0], np.float32).T).astype(bfloat16)
    tbl = _build_tbl(ng)
    ident = np.eye(128, dtype=bfloat16)

    nc = _get_program(ng)

    in_maps = []
    for h in range(N_CORES):
        wq = np.concatenate([w_q_sw[:, h, :], w_q_g[:, h, :]], axis=1)
        wk = np.concatenate([w_k_sw[:, h, :], w_k_g[:, h, :]], axis=1)
        wv = np.concatenate([w_v_sw[:, h, :], w_v_g[:, h, :]], axis=1)
        wqkv = np.concatenate([wq, wk, wv], axis=1).astype(bfloat16)
        b3 = np.stack([np.concatenate([b_q_sw[h], b_q_g[h]]).reshape(-1),
                       np.concatenate([b_k_sw[h], b_k_g[h]]).reshape(-1),
                       np.concatenate([b_v_sw[h], b_v_g[h]]).reshape(-1)],
                      axis=1).astype(np.float32)
        in_maps.append({
            "xqT": xqT, "xkvT": xkvT,
            "wqkv": wqkv, "b3": b3,
            "tbl": tbl, "ident": ident,
        })

    res = run_bass_kernel_spmd(nc, in_maps, list(range(N_CORES)),
                               trace=_trace, tmpdir=_tmpdir)
    out = np.zeros((S, F), np.float32)
    for h in range(N_CORES):
        xall = np.asarray(res.results[h]["xall"], dtype=np.float32)  # [65, S]
        xh = (xall[:HD] / xall[HD]).T                                # [S, 64]
        if ng > 0:
            xg = np.asarray(res.results[h]["xg"], dtype=np.float32)  # [65, ng]
            xh[:ng] = (xg[:HD] / xg[HD]).T
        out += xh @ np.asarray(w_out[h], np.float32)
    out += np.asarray(b_out, np.float32)
    if _trace:
        kernel._last_results = res
    return out[None].astype(np.float32)
